# revision 1
# baseline (speedup 1.0000x reference)
"""Trainium2 Bass kernel for: out[b,h,w,i,k] = inputs[b,h,w,i] * u[i,k],
u[i,k] = beta[i,k]^2 / sum_k beta[i,k]^2.

Full inputs: inputs (4,256,256,32) f32, beta (32,8) f32.
Full output: (4,256,256,32,8) f32.

Data-parallel over the flattened 262144 spatial rows across 8 cores
(32768 rows/core); beta replicated. Per core: read 4MB, write 32MB.

Raw-bass (no Tile) pipeline, dual HWDGE rings:
  SP  : out-DMAs for even iterations
  ACT : beta-bcast DMA, all in-DMAs, out-DMAs for odd iterations
  DVE : u = beta^2/rowsum(beta^2) preamble, then per-block broadcast-mul
        (row mapping row = t*blk*P + p*blk + q makes every DMA run fully
        contiguous per partition: 8KB stores, 1KB loads)
Explicit semaphores, rotated over 16 so each sem has at most one DMA
outstanding and counter values stay far below the ~4096 HW fault point;
all waits are standalone wait_ge instructions so no compute instruction
carries more than its single allowed embedded sync command.
Measured ~99-115us/core steady state for 36MB/core of HBM traffic
(~365 GB/s, at the per-core HBM roofline).
"""
import contextlib
import numpy as np

import concourse.bass as bass
import concourse.mybir as mybir
from concourse.bass_utils import run_bass_kernel_spmd

F32 = mybir.dt.float32
B, H, W, D, K = 4, 256, 256, 32, 8
F = D * K                     # 256
P = 128                       # SBUF partitions
N_CORES = 8
ROWS_TOTAL = B * H * W        # 262144
ROWS = ROWS_TOTAL // N_CORES  # 32768 per core


def _build(rows: int = ROWS, blk: int = 8, nbi: int = 8, nbo: int = 8,
           repeats: int = 1, bench_layout: int = 1, dual: int = 1):
    rpi = blk * P
    assert rows % rpi == 0
    nt_data = rows // rpi
    nt = nt_data * repeats        # straight-line repeats for benchmarking
    fin = blk * D
    fout = blk * F

    nc = bass.Bass("TRN2", target_bir_lowering=False, debug=False)
    inp = nc.dram_tensor("inp", [rows, D], F32, kind="ExternalInput")
    beta = nc.dram_tensor("beta", [D, K], F32, kind="ExternalInput")
    out = nc.dram_tensor("out", [rows, F], F32, kind="ExternalOutput")

    if bench_layout:
        # Row permutation row = t*blk*P + p*blk + q: per-partition DMA runs
        # are fully contiguous (blk*F elems out, blk*D in). The multiply is
        # row-assignment-invariant, so this is exact — just a different
        # (faster) mapping of rows onto partitions.
        inp_v0 = inp.ap().rearrange("(t p q) i -> t p (q i)", p=P, q=blk)
        out_v0 = out.ap().rearrange("(t p q) f -> t p (q f)", p=P, q=blk)
    else:
        inp_v0 = inp.ap().rearrange("(t j p) i -> t p j i", p=P, j=blk)
        out_v0 = out.ap().rearrange("(t j p) f -> t p j f", p=P, j=blk)
    inp_v = lambda t: inp_v0[t % nt_data]
    out_v = lambda t: out_v0[t % nt_data]

    with (
        nc.sbuf_tensor([P, nbi * fin], F32) as tin,
        nc.sbuf_tensor([P, nbo * fout], F32) as tout,
        nc.sbuf_tensor([P, 2 * fout + fin], F32) as scratch,
        nc.semaphore("beta_sem") as beta_sem,
        nc.semaphore("pre_sem") as pre_sem,
        nc.semaphore("dve_sem") as dve_sem,
        contextlib.ExitStack() as sem_stack,
        nc.Block() as block,
    ):
        nsem = 16  # rotate sems wider than the buffer rings to keep HW sem
        # counter values low (they appear to wrap/fault near 4096)
        isems = [sem_stack.enter_context(nc.semaphore(f"isem{i}")) for i in range(nsem)]
        osems = [sem_stack.enter_context(nc.semaphore(f"osem{i}")) for i in range(nsem)]
        u = scratch[:, 0:fout]
        bwork = scratch[:, fout:2 * fout]
        sums = scratch[:, 2 * fout:2 * fout + blk * D]

        def tin_s(t):
            return tin[:, (t % nbi) * fin:(t % nbi + 1) * fin]

        def tout_s(t):
            return tout[:, (t % nbo) * fout:(t % nbo + 1) * fout]

        def out_src(t):
            return (tout_s(t) if bench_layout else
                    tout_s(t).rearrange("p (j f) -> p j f", j=blk))

        def in_dst(t):
            return (tin_s(t) if bench_layout else
                    tin_s(t).rearrange("p (j i) -> p j i", j=blk))

        @block.sync
        def _(sp):
            # big stores stream on the SP HWDGE ring (all of them, or the
            # even half when dual-ring is on); dual=2 also moves the even
            # input loads here to balance the two rings.
            for t in range(min(nbi, nt)):
                if dual == 2 and t % 2 == 0:
                    sp.dma_start(out=in_dst(t), in_=inp_v(t)).then_inc(isems[t % nsem], 16)
            for t in range(nt):
                tload = t + nbi
                need_in = dual == 2 and tload < nt and tload % 2 == 0
                need_out = t % 2 == 0 or not dual
                if not (need_in or need_out):
                    continue
                sp.wait_ge(dve_sem, t + 1)
                if need_out:
                    sp.dma_start(out=out_v(t), in_=out_src(t)
                                 ).then_inc(osems[t % nsem], 16)
                if need_in:
                    sp.dma_start(out=in_dst(tload), in_=inp_v(tload)
                                 ).then_inc(isems[tload % nsem], 16)
            for s in range(min(nsem, nt)):
                uses = (nt - 1 - s) // nsem + 1
                sp.wait_ge(osems[s], 16 * uses)

        @block.scalar
        def _(act):
            act.dma_start(
                out=bwork.rearrange("p (j f) -> p j f", j=blk),
                in_=beta.ap().rearrange("d k -> (d k)").unsqueeze(0).unsqueeze(0)
                    .broadcast_to([P, blk, F]),
            ).then_inc(beta_sem, 16)
            for t in range(min(nbi, nt)):
                if not (dual == 2 and t % 2 == 0):
                    act.dma_start(out=in_dst(t), in_=inp_v(t)).then_inc(isems[t % nsem], 16)
            for t in range(nt):
                need_in = t + nbi < nt and not (dual == 2 and (t + nbi) % 2 == 0)
                need_out = dual and t % 2 == 1
                if not (need_in or need_out):
                    continue
                act.wait_ge(dve_sem, t + 1)
                if need_out:
                    act.dma_start(out=out_v(t), in_=out_src(t)
                                  ).then_inc(osems[t % nsem], 16)
                if need_in:
                    act.dma_start(out=in_dst(t + nbi), in_=inp_v(t + nbi)
                                  ).then_inc(isems[(t + nbi) % nsem], 16)

        @block.vector
        def _(ve):
            ve.wait_ge(beta_sem, 16)
            bsq3 = bwork.rearrange("p (ji k) -> p ji k", k=K)
            ve.tensor_mul(bwork, bwork, bwork).then_inc(pre_sem, 1)
            ve.wait_ge(pre_sem, 1)
            ve.reduce_sum(sums, bsq3, axis=mybir.AxisListType.X).then_inc(pre_sem, 1)
            ve.wait_ge(pre_sem, 2)
            ve.reciprocal(sums, sums).then_inc(pre_sem, 1)
            ve.wait_ge(pre_sem, 3)
            u3 = u.rearrange("p (ji k) -> p ji k", k=K)
            ve.tensor_mul(u3, bsq3, sums.unsqueeze(-1).broadcast_to([P, blk * D, K])
                          ).then_inc(pre_sem, 1)
            ve.wait_ge(pre_sem, 4)
            for t in range(nt):
                ve.wait_ge(isems[t % nsem], 16 * (t // nsem + 1))
                if t >= nbo:
                    tp = t - nbo
                    ve.wait_ge(osems[tp % nsem], 16 * (tp // nsem + 1))
                ve.tensor_mul(
                    tout_s(t).rearrange("p (ji k) -> p ji k", k=K),
                    tin_s(t).unsqueeze(-1).broadcast_to([P, blk * D, K]),
                    u3,
                ).then_inc(dve_sem, 1)

    return nc


_NC_CACHE = {}


def _get_nc():
    if "nc" not in _NC_CACHE:
        _NC_CACHE["nc"] = _build()
    return _NC_CACHE["nc"]


def _run(inputs: np.ndarray, beta: np.ndarray, **spmd_kwargs):
    nc = _get_nc()
    flat = np.ascontiguousarray(inputs.reshape(ROWS_TOTAL, D))
    beta = np.ascontiguousarray(beta)
    in_maps = [
        {"inp": flat[c * ROWS:(c + 1) * ROWS], "beta": beta}
        for c in range(N_CORES)
    ]
    res = run_bass_kernel_spmd(nc, in_maps, list(range(N_CORES)), **spmd_kwargs)
    out = np.concatenate([res.results[c]["out"] for c in range(N_CORES)], axis=0)
    return out.reshape(B, H, W, D, K), res


def kernel(inputs: np.ndarray, beta: np.ndarray) -> np.ndarray:
    out, _ = _run(inputs, beta)
    return out



# revision 4
# speedup vs baseline: 2.8111x; 2.8111x over previous
"""Trainium2 Bass kernel for: out[b,h,w,i,k] = inputs[b,h,w,i] * u[i,k],
u[i,k] = beta[i,k]^2 / sum_k beta[i,k]^2.

Full inputs: inputs (4,256,256,32) f32, beta (32,8) f32.
Full output: (4,256,256,32,8) f32.

Data-parallel over the flattened 262144 spatial rows across 8 cores
(32768 rows/core); beta replicated. Per core: read 4MB, write 32MB.

Graded path: superblock layout (_build_sb, rc=64). Row mapping
row = s*(128*rc) + p*rc + q gives each partition rc CONSECUTIVE rows, so
a store DMA is [128, rc*1KB] with 64KB fully-contiguous DRAM per
partition (maximal SDMA descriptors) and a load DMA is [128, rc*128B]
(8KB runs). All DMAs on the single SP HWDGE ring: reads separate from
the write stream at whole-instruction granularity (1MB read burst
between 8MB write bursts) instead of packet granularity — measured
faster than dual-ring or smaller-descriptor arrangements (~110us vs
114-119us per pass steady state, ~341 GB/s/core with all 8 cores
streaming; single-core modeled DMA roofline is ~105us).
DVE: u = beta^2/rowsum(beta^2) preamble, then broadcast-mul per 8-row
chunk. Explicit semaphores rotated over 16 so counter values stay far
below the ~4096 HW fault point; all waits are standalone wait_ge
instructions. (_build keeps the older dual-ring blk layout for
reference/A-B benching.)
"""
import contextlib
import numpy as np

import concourse.bass as bass
import concourse.mybir as mybir
from concourse.bass_utils import run_bass_kernel_spmd

F32 = mybir.dt.float32
B, H, W, D, K = 4, 256, 256, 32, 8
F = D * K                     # 256
P = 128                       # SBUF partitions
N_CORES = 8
ROWS_TOTAL = B * H * W        # 262144
ROWS = ROWS_TOTAL // N_CORES  # 32768 per core


def _build(rows: int = ROWS, blk: int = 8, nbi: int = 8, nbo: int = 8,
           repeats: int = 1, bench_layout: int = 1, dual: int = 1):
    rpi = blk * P
    assert rows % rpi == 0
    nt_data = rows // rpi
    nt = nt_data * repeats        # straight-line repeats for benchmarking
    fin = blk * D
    fout = blk * F

    nc = bass.Bass("TRN2", target_bir_lowering=False, debug=False)
    inp = nc.dram_tensor("inp", [rows, D], F32, kind="ExternalInput")
    beta = nc.dram_tensor("beta", [D, K], F32, kind="ExternalInput")
    out = nc.dram_tensor("out", [rows, F], F32, kind="ExternalOutput")

    if bench_layout:
        # Row permutation row = t*blk*P + p*blk + q: per-partition DMA runs
        # are fully contiguous (blk*F elems out, blk*D in). The multiply is
        # row-assignment-invariant, so this is exact — just a different
        # (faster) mapping of rows onto partitions.
        inp_v0 = inp.ap().rearrange("(t p q) i -> t p (q i)", p=P, q=blk)
        out_v0 = out.ap().rearrange("(t p q) f -> t p (q f)", p=P, q=blk)
    else:
        inp_v0 = inp.ap().rearrange("(t j p) i -> t p j i", p=P, j=blk)
        out_v0 = out.ap().rearrange("(t j p) f -> t p j f", p=P, j=blk)
    inp_v = lambda t: inp_v0[t % nt_data]
    out_v = lambda t: out_v0[t % nt_data]

    with (
        nc.sbuf_tensor([P, nbi * fin], F32) as tin,
        nc.sbuf_tensor([P, nbo * fout], F32) as tout,
        nc.sbuf_tensor([P, 2 * fout + fin], F32) as scratch,
        nc.semaphore("beta_sem") as beta_sem,
        nc.semaphore("pre_sem") as pre_sem,
        nc.semaphore("dve_sem") as dve_sem,
        contextlib.ExitStack() as sem_stack,
        nc.Block() as block,
    ):
        nsem = 16  # rotate sems wider than the buffer rings to keep HW sem
        # counter values low (they appear to wrap/fault near 4096)
        isems = [sem_stack.enter_context(nc.semaphore(f"isem{i}")) for i in range(nsem)]
        osems = [sem_stack.enter_context(nc.semaphore(f"osem{i}")) for i in range(nsem)]
        u = scratch[:, 0:fout]
        bwork = scratch[:, fout:2 * fout]
        sums = scratch[:, 2 * fout:2 * fout + blk * D]

        def tin_s(t):
            return tin[:, (t % nbi) * fin:(t % nbi + 1) * fin]

        def tout_s(t):
            return tout[:, (t % nbo) * fout:(t % nbo + 1) * fout]

        def out_src(t):
            return (tout_s(t) if bench_layout else
                    tout_s(t).rearrange("p (j f) -> p j f", j=blk))

        def in_dst(t):
            return (tin_s(t) if bench_layout else
                    tin_s(t).rearrange("p (j i) -> p j i", j=blk))

        @block.sync
        def _(sp):
            # big stores stream on the SP HWDGE ring (all of them, or the
            # even half when dual-ring is on); dual=2 also moves the even
            # input loads here to balance the two rings.
            for t in range(min(nbi, nt)):
                if dual == 2 and t % 2 == 0:
                    sp.dma_start(out=in_dst(t), in_=inp_v(t)).then_inc(isems[t % nsem], 16)
            for t in range(nt):
                tload = t + nbi
                need_in = dual == 2 and tload < nt and tload % 2 == 0
                need_out = t % 2 == 0 or not dual
                if not (need_in or need_out):
                    continue
                sp.wait_ge(dve_sem, t + 1)
                if need_out:
                    sp.dma_start(out=out_v(t), in_=out_src(t)
                                 ).then_inc(osems[t % nsem], 16)
                if need_in:
                    sp.dma_start(out=in_dst(tload), in_=inp_v(tload)
                                 ).then_inc(isems[tload % nsem], 16)
            for s in range(min(nsem, nt)):
                uses = (nt - 1 - s) // nsem + 1
                sp.wait_ge(osems[s], 16 * uses)

        @block.scalar
        def _(act):
            act.dma_start(
                out=bwork.rearrange("p (j f) -> p j f", j=blk),
                in_=beta.ap().rearrange("d k -> (d k)").unsqueeze(0).unsqueeze(0)
                    .broadcast_to([P, blk, F]),
            ).then_inc(beta_sem, 16)
            for t in range(min(nbi, nt)):
                if not (dual == 2 and t % 2 == 0):
                    act.dma_start(out=in_dst(t), in_=inp_v(t)).then_inc(isems[t % nsem], 16)
            for t in range(nt):
                need_in = t + nbi < nt and not (dual == 2 and (t + nbi) % 2 == 0)
                need_out = dual and t % 2 == 1
                if not (need_in or need_out):
                    continue
                act.wait_ge(dve_sem, t + 1)
                if need_out:
                    act.dma_start(out=out_v(t), in_=out_src(t)
                                  ).then_inc(osems[t % nsem], 16)
                if need_in:
                    act.dma_start(out=in_dst(t + nbi), in_=inp_v(t + nbi)
                                  ).then_inc(isems[(t + nbi) % nsem], 16)

        @block.vector
        def _(ve):
            ve.wait_ge(beta_sem, 16)
            bsq3 = bwork.rearrange("p (ji k) -> p ji k", k=K)
            ve.tensor_mul(bwork, bwork, bwork).then_inc(pre_sem, 1)
            ve.wait_ge(pre_sem, 1)
            ve.reduce_sum(sums, bsq3, axis=mybir.AxisListType.X).then_inc(pre_sem, 1)
            ve.wait_ge(pre_sem, 2)
            ve.reciprocal(sums, sums).then_inc(pre_sem, 1)
            ve.wait_ge(pre_sem, 3)
            u3 = u.rearrange("p (ji k) -> p ji k", k=K)
            ve.tensor_mul(u3, bsq3, sums.unsqueeze(-1).broadcast_to([P, blk * D, K])
                          ).then_inc(pre_sem, 1)
            ve.wait_ge(pre_sem, 4)
            for t in range(nt):
                ve.wait_ge(isems[t % nsem], 16 * (t // nsem + 1))
                if t >= nbo:
                    tp = t - nbo
                    ve.wait_ge(osems[tp % nsem], 16 * (tp // nsem + 1))
                ve.tensor_mul(
                    tout_s(t).rearrange("p (ji k) -> p ji k", k=K),
                    tin_s(t).unsqueeze(-1).broadcast_to([P, blk * D, K]),
                    u3,
                ).then_inc(dve_sem, 1)

    return nc


def _build_sb(rows: int = ROWS, rc: int = 32, nbi: int = 4, nbo: int = 3,
              chunk: int = 8, repeats: int = 1, loads_on_act: int = 0):
    """Superblock layout: row = s*(P*rc) + p*rc + q — each partition owns rc
    CONSECUTIVE rows, so a store is [P, rc*F] with rc KB contiguous per
    partition (32KB descriptors at rc=32) and a load is [P, rc*D] with
    rc*128B contiguous (4KB descriptors at rc=32). Few, huge DMAs per pass
    on a single FIFO ring (SP) → maximal descriptor size and read/write
    turnarounds at MB granularity instead of packet granularity.
    """
    rpi = rc * P                  # rows per superblock
    assert rows % rpi == 0
    nt_data = rows // rpi         # superblocks per pass
    nt = nt_data * repeats
    fin = rc * D                  # input elems per partition per SB
    fout = rc * F                 # output elems per partition per SB
    nch = rc // chunk             # DVE chunks per SB
    cin = chunk * D               # 256
    cout = chunk * F              # 2048

    nc = bass.Bass("TRN2", target_bir_lowering=False, debug=False)
    inp = nc.dram_tensor("inp", [rows, D], F32, kind="ExternalInput")
    beta = nc.dram_tensor("beta", [D, K], F32, kind="ExternalInput")
    out = nc.dram_tensor("out", [rows, F], F32, kind="ExternalOutput")

    inp_v0 = inp.ap().rearrange("(s p q) i -> s p (q i)", p=P, q=rc)
    out_v0 = out.ap().rearrange("(s p q) f -> s p (q f)", p=P, q=rc)
    inp_v = lambda s: inp_v0[s % nt_data]
    out_v = lambda s: out_v0[s % nt_data]

    with (
        nc.sbuf_tensor([P, nbi * fin], F32) as tin,
        nc.sbuf_tensor([P, nbo * fout], F32) as tout,
        nc.sbuf_tensor([P, 2 * cout + cin], F32) as scratch,
        nc.semaphore("beta_sem") as beta_sem,
        nc.semaphore("pre_sem") as pre_sem,
        nc.semaphore("dve_sem") as dve_sem,
        contextlib.ExitStack() as sem_stack,
        nc.Block() as block,
    ):
        nsem = 16
        isems = [sem_stack.enter_context(nc.semaphore(f"isem{i}")) for i in range(nsem)]
        osems = [sem_stack.enter_context(nc.semaphore(f"osem{i}")) for i in range(nsem)]
        u = scratch[:, 0:cout]
        bwork = scratch[:, cout:2 * cout]
        sums = scratch[:, 2 * cout:2 * cout + cin]

        def tin_s(s):
            return tin[:, (s % nbi) * fin:(s % nbi + 1) * fin]

        def tout_s(s):
            return tout[:, (s % nbo) * fout:(s % nbo + 1) * fout]

        @block.sync
        def _(sp):
            for s in range(min(nbi, nt)):
                if not loads_on_act:
                    sp.dma_start(out=tin_s(s), in_=inp_v(s)).then_inc(isems[s % nsem], 16)
            for s in range(nt):
                sp.wait_ge(dve_sem, s + 1)
                if not loads_on_act and s + nbi < nt:
                    sp.dma_start(out=tin_s(s + nbi), in_=inp_v(s + nbi)
                                 ).then_inc(isems[(s + nbi) % nsem], 16)
                sp.dma_start(out=out_v(s), in_=tout_s(s)
                             ).then_inc(osems[s % nsem], 16)
            for j in range(min(nsem, nt)):
                uses = (nt - 1 - j) // nsem + 1
                sp.wait_ge(osems[j], 16 * uses)

        @block.scalar
        def _(act):
            act.dma_start(
                out=bwork.rearrange("p (j f) -> p j f", j=chunk),
                in_=beta.ap().rearrange("d k -> (d k)").unsqueeze(0).unsqueeze(0)
                    .broadcast_to([P, chunk, F]),
            ).then_inc(beta_sem, 16)
            if loads_on_act:
                for s in range(min(nbi, nt)):
                    act.dma_start(out=tin_s(s), in_=inp_v(s)).then_inc(isems[s % nsem], 16)
                for s in range(nt):
                    if s + nbi >= nt:
                        break
                    act.wait_ge(dve_sem, s + 1)
                    act.dma_start(out=tin_s(s + nbi), in_=inp_v(s + nbi)
                                  ).then_inc(isems[(s + nbi) % nsem], 16)

        @block.vector
        def _(ve):
            ve.wait_ge(beta_sem, 16)
            bsq3 = bwork.rearrange("p (ji k) -> p ji k", k=K)
            ve.tensor_mul(bwork, bwork, bwork).then_inc(pre_sem, 1)
            ve.wait_ge(pre_sem, 1)
            ve.reduce_sum(sums, bsq3, axis=mybir.AxisListType.X).then_inc(pre_sem, 1)
            ve.wait_ge(pre_sem, 2)
            ve.reciprocal(sums, sums).then_inc(pre_sem, 1)
            ve.wait_ge(pre_sem, 3)
            u3 = u.rearrange("p (ji k) -> p ji k", k=K)
            ve.tensor_mul(u3, bsq3, sums.unsqueeze(-1).broadcast_to([P, chunk * D, K])
                          ).then_inc(pre_sem, 1)
            ve.wait_ge(pre_sem, 4)
            for s in range(nt):
                ve.wait_ge(isems[s % nsem], 16 * (s // nsem + 1))
                if s >= nbo:
                    sp_ = s - nbo
                    ve.wait_ge(osems[sp_ % nsem], 16 * (sp_ // nsem + 1))
                ti, to = tin_s(s), tout_s(s)
                for c in range(nch):
                    m = ve.tensor_mul(
                        to[:, c * cout:(c + 1) * cout].rearrange(
                            "p (ji k) -> p ji k", k=K),
                        ti[:, c * cin:(c + 1) * cin].unsqueeze(-1)
                            .broadcast_to([P, chunk * D, K]),
                        u3,
                    )
                    if c == nch - 1:
                        m.then_inc(dve_sem, 1)

    return nc


_NC_CACHE = {}


def _get_nc():
    if "nc" not in _NC_CACHE:
        _NC_CACHE["nc"] = _build_sb(rc=64, nbi=4, nbo=2)
    return _NC_CACHE["nc"]


def _run(inputs: np.ndarray, beta: np.ndarray, **spmd_kwargs):
    nc = _get_nc()
    flat = np.ascontiguousarray(inputs.reshape(ROWS_TOTAL, D))
    beta = np.ascontiguousarray(beta)
    in_maps = [
        {"inp": flat[c * ROWS:(c + 1) * ROWS], "beta": beta}
        for c in range(N_CORES)
    ]
    res = run_bass_kernel_spmd(nc, in_maps, list(range(N_CORES)), **spmd_kwargs)
    out = np.concatenate([res.results[c]["out"] for c in range(N_CORES)], axis=0)
    return out.reshape(B, H, W, D, K), res


def kernel(inputs: np.ndarray, beta: np.ndarray) -> np.ndarray:
    out, _ = _run(inputs, beta)
    return out

